# revision 14
# baseline (speedup 1.0000x reference)
"""Trainium2 Bass kernel for nn_LinearTextEmbedding.

out[n, c, x, y] = 1.0 if |bits[n, (512*x + y) % 1024]| > 0.5 else 0.0

Key structure: the flattened 512*512 map is the 1024-element thresholded
bit pattern tiled 256 times, and all 16 channels are identical, so the
kernel is pure HBM-write bandwidth (64 MiB per core).  Pipeline:

1. ONE 32 KiB load per execution: bits for all 4 samples, each sample's
   1024 f32 duplicated (stride-0 dim) -> [1, 8192] = [t0 t0 .. t3 t3].
2. Threshold on that single partition (x*x > 0.25; abs_max fails the
   TRN2 TensorScalar ISA check, squaring is exact for the comparison).
3. Partition-broadcast each sample's 2048-wide row to 128 partitions on
   the (otherwise idle) TensorEngine: ones[1,128].T @ row[1,512] x 4
   into a 4-bank PSUM tile (matmul moving free dim is capped at 512),
   then one DVE copy PSUM -> SBUF.  Broadcasting on-chip instead of a
   stride-0 DMA load matters far beyond its 4 MiB of traffic: the 256x
   re-read of the same 4 KiB fights the write streams for HBM (measured
   222 -> 174 us/exec when removed).
4. Store each sample's 16 channels as TWO 8 MiB DMAs (8 channels each,
   stride-0 middle dim re-reads the tile per channel) on the two HWDGE
   rings.  Splitting within the sample keeps both rings evenly loaded
   and halves the final drain tail (measured 223 -> 208 us vs
   alternating whole samples across rings).

Per-core steady state: ~174 us for 64 MiB written = ~385 GB/s.

Sharding: pure data parallel, 32 samples -> 8 cores x 4 samples.
"""

from contextlib import ExitStack

import numpy as np

import concourse.bass as bass
import concourse.bacc as bacc
import concourse.mybir as mybir
import concourse.tile as tile
from concourse.bass_utils import run_bass_kernel_spmd

F32 = mybir.dt.float32

B = 32          # full batch
NBITS = 1024
NCORES = 8
BPC = B // NCORES   # samples per core
CH = 16
W = H = 512
MAP = W * H         # 262144 = 256 repeats of the 1024 pattern
REP_COLS = 2048     # 2 copies of the pattern per partition
# store queue allocation per sample: (first channel, n channels, engine)
STORE_SPLITS = [(0, 6, "sync"), (6, 5, "scalar"), (11, 5, "gpsimd")]
MM_N = 512          # matmul moving free dim cap (one PSUM bank of f32)
# (128 partitions) x (2048 f32) = 262144 elements = one full channel map.
# Partition p holds map elements [p*2048, (p+1)*2048) = rows 4p..4p+3,
# which is [t0 t1 t0 t1] (t0=pattern[0:512] even rows, t1=pattern[512:1024]
# odd rows) -> identical content in every partition.

_NC_CACHE = {}


def _build(reps=1, internal_out=False):
    """Trace the kernel body `reps` times into one module.

    reps=1, internal_out=False is the graded kernel.  The other
    configurations exist only for timing (test.py): a NEFF that does the
    identical device work N times lets a wall-clock bench recover
    per-execution HW time as the slope between two rep counts, cancelling
    the (large, noisy) fixed launch overhead of the axon tunnel.  Every
    rep writes the same values to `out`, so the result is unchanged.
    internal_out=True additionally makes `out` an Internal DRAM tensor
    (same HBM writes, same device work) with an 8 KiB `chk` output, so a
    bench launch doesn't allocate/transfer 512 MiB — that allocation adds
    15-45 ms of launch-to-launch wall-clock noise that otherwise swamps
    the slope.
    """
    nc = bacc.Bacc(None, target_bir_lowering=False)
    bits = nc.dram_tensor("bits", [BPC, NBITS], F32, kind="ExternalInput")
    out = nc.dram_tensor(
        "out", [BPC, CH, MAP], F32,
        kind="Internal" if internal_out else "ExternalOutput",
    )
    chk = (
        nc.dram_tensor("chk", [1, REP_COLS], F32, kind="ExternalOutput")
        if internal_out else None
    )

    with tile.TileContext(nc) as tc:
        with ExitStack() as ctx:
            # bufs=6: all four sample tiles live at once (8 KiB/partition
            # each), so a store never WAR-stalls the next sample's fill
            # and the two HWDGE rings stay back-to-back busy.  A single
            # execution allocates only 4 "rep" tiles; the extra depth
            # lets the reps>1 bench twin pipeline across rep boundaries
            # the way back-to-back dispatches would.
            pool = ctx.enter_context(tc.tile_pool(name="pool", bufs=6))
            # [128, 2048] f32 PSUM tile = 4 banks; bufs=2 uses all 8.
            psp = ctx.enter_context(tc.psum_pool(name="ps", bufs=2))
            ones = pool.tile([1, 128], F32, name="ones", bufs=1)
            nc.vector.memset(ones[:], 1.0)
            def _load_smallf():
                # [t0 t0 t1 t1 t2 t2 t3 t3]: all samples' bits, each
                # duplicated via a stride-0 mid dim (32 KiB total read).
                smallf = pool.tile([1, 8192], F32, name="smallf", bufs=2)
                nc.gpsimd.dma_start(
                    smallf[:],
                    bass.AP(bits, 0, [[NBITS, BPC], [0, 2], [1, NBITS]]),
                )
                return smallf

            # Software-pipeline the load across reps: rep r+1's load is
            # issued at the TOP of rep r, so in the gpsimd FIFO it sits
            # AHEAD of rep r's 20 MiB of channel stores instead of
            # draining after them and gating the next rep's fill chain.
            # For reps=1 this emits the identical single-load program.
            smallf_next = _load_smallf()
            for r in range(reps):
                smallf = smallf_next
                if r + 1 < reps:
                    smallf_next = _load_smallf()
                small = pool.tile([1, 8192], F32, name="small", bufs=2)
                nc.vector.tensor_mul(smallf[:], smallf[:], smallf[:])
                nc.vector.tensor_scalar(
                    small[:], smallf[:], 0.25, None,
                    op0=mybir.AluOpType.is_gt,
                )
                for s in range(BPC):
                    # TensorE partition-broadcast: ones.T @ row, 512
                    # columns per matmul (PSUM bank), 4 banks = 2048.
                    ps = psp.tile([128, REP_COLS], F32, name="psb")
                    for k in range(REP_COLS // MM_N):
                        lo = REP_COLS * s + MM_N * k
                        nc.tensor.matmul(
                            out=ps[:, MM_N * k:MM_N * (k + 1)],
                            lhsT=ones[:],
                            rhs=small[:, lo:lo + MM_N],
                            start=True, stop=True,
                        )
                    rep = pool.tile([128, REP_COLS], F32, name="rep")
                    nc.vector.tensor_copy(rep[:], ps[:])
                    # The sample's 16 channels x 1 MiB go out as THREE
                    # DMAs on three queues — both HWDGE rings plus the
                    # (otherwise idle) SWDGE queue — each with a stride-0
                    # middle dim re-reading the tile per channel.  With
                    # three streams, one queue's completion handshake
                    # hides behind the other two and the SBUF ports stay
                    # busy (measured 196 -> 173 us vs the 8/8 two-ring
                    # split, same session).
                    for c0, n, eng in STORE_SPLITS:
                        src_store = rep[:].unsqueeze(1).broadcast_to(
                            (128, n, REP_COLS)
                        )
                        dst = bass.AP(
                            out, (s * CH + c0) * MAP,
                            [[REP_COLS, 128], [MAP, n], [1, REP_COLS]],
                        )
                        getattr(nc, eng).dma_start(dst, src_store)
                    if chk is not None and r == reps - 1 and s == 0:
                        # tiny live output so the bench NEFF has a
                        # non-Internal result (and a correctness probe:
                        # partition 0 of the broadcast thresholded tile).
                        nc.sync.dma_start(
                            bass.AP(
                                chk, 0, [[REP_COLS, 1], [1, REP_COLS]]
                            ),
                            bass.AP(
                                rep.tensor, rep[:].offset,
                                [[REP_COLS, 1], [1, REP_COLS]],
                            ),
                        )
    return nc


def _get_nc(reps=1, internal_out=False):
    key = (reps, internal_out)
    if key not in _NC_CACHE:
        nc = _build(reps, internal_out)
        # run_bass_via_pjrt serializes nc.m as-is; Bacc defers register
        # allocation to finalize(), so finalize here or walrus sees
        # unallocated registers.
        nc.finalize()
        _NC_CACHE[key] = nc
    return _NC_CACHE[key]


def run_sharded(bits: np.ndarray, **spmd_kwargs):
    """Run on 8 cores; returns (full_output, BassKernelResults)."""
    nc = _get_nc()
    bits = np.ascontiguousarray(np.asarray(bits, dtype=np.float32))
    assert bits.shape == (B, NBITS), bits.shape
    in_maps = [
        {"bits": bits[k * BPC:(k + 1) * BPC]} for k in range(NCORES)
    ]
    res = run_bass_kernel_spmd(nc, in_maps, list(range(NCORES)), **spmd_kwargs)
    outs = [
        np.asarray(res.results[k]["out"]).reshape(BPC, CH, W, H)
        for k in range(NCORES)
    ]
    return np.concatenate(outs, axis=0), res


def kernel(bits: np.ndarray) -> np.ndarray:
    full, _ = run_sharded(bits)
    return full
